# revision 2
# baseline (speedup 1.0000x reference)
"""Trainium2 Bass kernel for nn_DeformableGCN (GNN message passing).

Strategy (1D graph partitioning over 8 NeuronCores):
  - Destination nodes are assigned to cores/tiles via a degree-sorted
    permutation pi: each 128-node tile holds nodes with nearly equal
    in-degree, so each dst node's in-edges occupy its own SBUF partition
    across a minimal number of 128-edge chunks ("identity scatter": the
    segment-sum matmul uses a constant identity weight matrix).
  - Per-edge source rows are fetched with the custom dma_gather
    instruction (int16 indices, 4 SWDGE queues). The node table is
    addressed through two OVERLAPPING 32768-row windows (lo = rows
    [0, 32768), hi = rows [NP-32768, NP)); sources in the overlap region
    may fill either stream, so a tile's chunk count approaches its max
    total in-degree instead of max-lo + max-hi. High-out-degree nodes
    are permuted into the overlap region to maximize flexible edges.
  - Each smoothing step computes the core's dst shard, then an AllGather
    rebuilds the full node table for the next step's gathers. The conv
    layers gather rows [h@W_lin | u_src] of per-node projected tables
    (AllGather'd); conv1's table is bf16 (a 256B descriptor carries the
    whole 65-value row vs 2x256B in f32), halving that pass's HBM and
    collective traffic. Edge scores lrelu(u_src + a_dst) use the
    per-tile a_dst column, partition-aligned by construction.
"""
import os
import sys

sys.path.insert(0, "/opt/trn_rl_repo")

import numpy as np

import concourse.bass as bass
import concourse.bacc as bacc
import concourse.mybir as mybir
import concourse.tile as tile
from concourse.masks import make_identity

M = 8            # cores
P = 128          # partitions
WLEN = 32768     # gather window length (int16-addressable rows)
WIN = 32         # gather-call window, in 128-edge chunks
F32 = mybir.dt.float32
BF16 = mybir.dt.bfloat16
I16 = mybir.dt.int16
NEG_SLOPE = 0.01


# ------------------------------------------------------------- pjrt runner

class _Runner:
    """Builds the jitted PJRT callable once; repeated exec without retrace."""

    def __init__(self, nc, n_cores):
        import jax
        from jax.sharding import Mesh, PartitionSpec
        from jax.experimental.shard_map import shard_map
        from concourse.bass2jax import (
            install_neuronx_cc_hook, _bass_exec_p, partition_id_tensor)
        install_neuronx_cc_hook()
        self.jax = jax
        self.n_cores = n_cores
        in_names, out_names, out_avals, zero_outs = [], [], [], []
        partition_name = (nc.partition_id_tensor.name
                          if nc.partition_id_tensor else None)
        for alloc in nc.m.functions[0].allocations:
            if not isinstance(alloc, mybir.MemoryLocationSet):
                continue
            name = alloc.memorylocations[0].name
            if alloc.kind == "ExternalInput":
                if name != partition_name:
                    in_names.append(name)
            elif alloc.kind == "ExternalOutput":
                shape = tuple(alloc.tensor_shape)
                dtype = mybir.dt.np(alloc.dtype)
                out_names.append(name)
                out_avals.append(jax.core.ShapedArray(shape, dtype))
                zero_outs.append(np.zeros(shape, dtype))
        self.in_names, self.out_names = in_names, out_names
        self.zero_outs = zero_outs
        n_params = len(in_names)
        all_in_names = list(in_names) + list(out_names)
        if partition_name is not None:
            all_in_names.append(partition_name)

        def _body(*args):
            operands = list(args)
            if partition_name is not None:
                operands.append(partition_id_tensor())
            outs = _bass_exec_p.bind(
                *operands,
                out_avals=tuple(out_avals),
                in_names=tuple(all_in_names),
                out_names=tuple(out_names),
                lowering_input_output_aliases=(),
                sim_require_finite=True,
                sim_require_nnan=True,
                nc=nc,
            )
            return tuple(outs)

        donate = tuple(range(n_params, n_params + len(out_names)))
        devices = jax.devices()[:n_cores]
        self.mesh = Mesh(np.asarray(devices), ("core",))
        in_specs = (PartitionSpec("core"),) * (n_params + len(out_names))
        out_specs = (PartitionSpec("core"),) * len(out_names)
        self.fn = jax.jit(
            shard_map(_body, mesh=self.mesh, in_specs=in_specs,
                      out_specs=out_specs, check_rep=False),
            donate_argnums=donate, keep_unused=True)
        self._dev_inputs = None

    def place_inputs(self, in_maps):
        import jax
        from jax.sharding import PartitionSpec, NamedSharding
        per_core = [[np.asarray(m[n]) for n in self.in_names]
                    for m in in_maps]
        arrs = []
        for i, n in enumerate(self.in_names):
            concat = np.concatenate(
                [per_core[c][i] for c in range(self.n_cores)], axis=0)
            arrs.append(jax.device_put(
                concat, NamedSharding(self.mesh, PartitionSpec("core"))))
        for a in arrs:
            a.block_until_ready()
        self._dev_inputs = arrs

    def _zeros(self):
        return [np.zeros((self.n_cores * z.shape[0], *z.shape[1:]), z.dtype)
                for z in self.zero_outs]

    def exec_async(self):
        return self.fn(*self._dev_inputs, *self._zeros())

    def run(self, in_maps=None):
        if in_maps is not None:
            self.place_inputs(in_maps)
        outs = [np.asarray(o) for o in self.exec_async()]
        res = []
        for c in range(self.n_cores):
            d = {}
            for i, n in enumerate(self.out_names):
                per = outs[i].reshape(
                    (self.n_cores, outs[i].shape[0] // self.n_cores)
                    + outs[i].shape[1:])
                d[n] = per[c]
            res.append(d)
        return res

    def time_exec(self, k=8):
        import time
        o = self.exec_async()
        self.jax.block_until_ready(o)
        t0 = time.perf_counter()
        outs = [self.exec_async() for _ in range(k)]
        self.jax.block_until_ready(outs)
        return (time.perf_counter() - t0) / k


# ---------------------------------------------------------------- schedule

def _build_schedule(src, dst, n_nodes, ls_iters=200000):
    """Host-side graph partitioning: permutation, slots, gather indices.

    Three source regions by final position: R0 = rows [0, NP-32768)
    (lo-stream only), R1 = [NP-32768, 32768) (either stream),
    R2 = [32768, NP) (hi-stream only).
    """
    E = src.shape[0]
    NP = -(-n_nodes // (M * P)) * (M * P)
    if NP - n_nodes < 2:
        NP += M * P
    TPC = NP // (M * P)
    NTILE = M * TPC
    HI_BASE = NP - WLEN
    assert 0 <= HI_BASE and NP <= 2 * WLEN and HI_BASE % P == 0
    n_r0_tiles = HI_BASE // P
    n_r1_tiles = (min(WLEN, NP) - HI_BASE) // P
    n_r2_tiles = NTILE - n_r0_tiles - n_r1_tiles

    deg = np.bincount(dst, minlength=NP).astype(np.int64)
    out_deg = np.bincount(src, minlength=NP).astype(np.int64)

    # region membership pinned before building the permutation; R1 (flex)
    # gets the highest OUT-degree nodes plus one fake (zero) pad node
    region = np.zeros(NP, np.int8)
    fakes = np.arange(n_nodes, NP)
    order_out = np.argsort(-out_deg, kind="stable")
    r1q = n_r1_tiles * P
    r1_sel = [i for i in order_out[: r1q] if i != fakes[0]][: r1q - 1]
    r1_sel.append(fakes[0])
    r1_mask = np.zeros(NP, bool)
    r1_mask[np.array(r1_sel)] = True
    assert r1_mask.sum() == r1q
    region[r1_mask] = 1
    rest = np.flatnonzero(~r1_mask)
    region[rest[: n_r0_tiles * P]] = 0
    region[rest[n_r0_tiles * P:]] = 2
    pad_fake = int(fakes[0])

    scat = region[src]            # 0=must-lo, 1=flex, 2=must-hi
    a = np.bincount(dst[scat == 0], minlength=NP)
    c = np.bincount(dst[scat == 1], minlength=NP)
    b = np.bincount(dst[scat == 2], minlength=NP)

    # tiles: total-degree primary, snake on a, then on b
    def pool_tiles(ids):
        t = deg[ids]
        sa = np.where(t % 2 == 0, a[ids], -a[ids])
        sb = np.where((t + a[ids]) % 2 == 0, b[ids], -b[ids])
        o = ids[np.lexsort((-sb, -sa, -t))]
        return o.reshape(-1, P)

    all_tiles = [(r, t) for r in (0, 1, 2)
                 for t in pool_tiles(np.flatnonzero(region == r))]
    kinds = np.array([r for r, _ in all_tiles])
    mtot = np.array([deg[t].max() for _, t in all_tiles])
    ma = np.array([a[t].max() for _, t in all_tiles])
    mb = np.array([b[t].max() for _, t in all_tiles])
    tcost = np.maximum(mtot, ma + mb)

    # position slots (c, tau) carry fixed region labels
    pos_region = np.zeros(NTILE, np.int8)
    pos_region[n_r0_tiles: n_r0_tiles + n_r1_tiles] = 1
    pos_region[n_r0_tiles + n_r1_tiles:] = 2
    quota = [[[] for _ in range(TPC)] for _ in range(3)]
    for pos in range(NTILE):
        cc, tau = divmod(pos, TPC)
        quota[pos_region[pos]][tau].append(cc)

    def tau_cost(mt, mA, mB):
        return max(mt, mA + mB)

    # percentile-aligned construction
    ids_sorted = {r: list(np.flatnonzero(kinds == r)[
        np.argsort(-tcost[kinds == r], kind="stable")]) for r in (0, 1, 2)}
    members = [[[] for _ in range(TPC)] for _ in range(3)]
    ptr = {0: 0, 1: 0, 2: 0}
    for tau in range(TPC):
        for r in (0, 1, 2):
            n = len(quota[r][tau])
            members[r][tau] = ids_sorted[r][ptr[r]: ptr[r] + n]
            ptr[r] += n

    def pos_cost(t):
        tis = members[0][t] + members[1][t] + members[2][t]
        mt = max((mtot[i] for i in tis), default=0)
        mA = max((ma[i] for i in tis), default=0)
        mB = max((mb[i] for i in tis), default=0)
        return tau_cost(mt, mA, mB)

    # simulated annealing over same-region tile swaps
    rng = np.random.default_rng(0)
    cur = np.array([pos_cost(t) for t in range(TPC)], np.int64)
    T0, T1 = 2.0, 0.01
    for it in range(ls_iters):
        temp = T0 * (T1 / T0) ** (it / max(ls_iters - 1, 1))
        k = int(rng.integers(0, 3))
        p_, q_ = (int(v) for v in rng.integers(0, TPC, 2))
        if p_ == q_ or not members[k][p_] or not members[k][q_]:
            continue
        i = members[k][p_][int(rng.integers(len(members[k][p_])))]
        j = members[k][q_][int(rng.integers(len(members[k][q_])))]
        before = cur[p_] + cur[q_]
        members[k][p_].remove(i)
        members[k][q_].remove(j)
        members[k][p_].append(j)
        members[k][q_].append(i)
        np_, nq_ = pos_cost(p_), pos_cost(q_)
        d = (np_ + nq_) - before
        if d <= 0 or rng.random() < np.exp(-d / temp):
            cur[p_], cur[q_] = np_, nq_
        else:
            members[k][p_].remove(j)
            members[k][q_].remove(i)
            members[k][p_].append(i)
            members[k][q_].append(j)

    CLO = np.zeros(TPC, np.int64)
    CHI = np.zeros(TPC, np.int64)
    for tau in range(TPC):
        tis = members[0][tau] + members[1][tau] + members[2][tau]
        mt = max((mtot[i] for i in tis), default=0)
        mA = max((ma[i] for i in tis), default=0)
        mB = max((mb[i] for i in tis), default=0)
        cost = tau_cost(mt, mA, mB)
        CLO[tau] = cost - mB
        CHI[tau] = mB
        assert CLO[tau] >= mA
    TOTC = int((CLO + CHI).sum())

    pi = np.empty(NP, np.int64)
    for r in (0, 1, 2):
        for tau in range(TPC):
            cores = list(quota[r][tau])
            for ti in members[r][tau]:
                cc = cores.pop()
                pi[all_tiles[ti][1]] = (cc * TPC + tau) * P + np.arange(P)
    pi_src = pi[src]
    pi_dst = pi[dst]

    LO_PAD = int(pi[pad_fake])
    HI_PAD = LO_PAD - HI_BASE
    assert 0 <= LO_PAD < WLEN and 0 <= HI_PAD < WLEN

    # per-edge stream + chunk-slot assignment
    cat = scat.astype(np.int64)
    tau_e = (pi_dst % (TPC * P)) // P
    key = pi_dst * 3 + cat
    eorder = np.argsort(key, kind="stable")
    ks = key[eorder]
    new_grp = np.ones(E, bool)
    new_grp[1:] = ks[1:] != ks[:-1]
    starts = np.flatnonzero(new_grp)
    grp_id = np.cumsum(new_grp) - 1
    rank = np.arange(E) - starts[grp_id]

    d_o = pi_dst[eorder]
    cat_o = cat[eorder]
    tau_o = tau_e[eorder]
    a_o = a[dst[eorder]]
    c_o = c[dst[eorder]]
    x_o = np.minimum(c_o, CLO[tau_o] - a_o)   # flex edges sent to lo
    is_lo_e = (cat_o == 0) | ((cat_o == 1) & (rank < x_o))
    lo_rank = np.where(cat_o == 0, rank, a_o + rank)
    hi_rank = np.where(cat_o == 1, rank - x_o,
                       np.maximum(c_o - x_o, 0) + rank)
    kchunk = np.where(is_lo_e, lo_rank, CLO[tau_o] + hi_rank)
    assert (np.where(is_lo_e, kchunk < CLO[tau_o],
                     kchunk < CLO[tau_o] + CHI[tau_o])).all()
    assert (kchunk >= np.where(is_lo_e, 0, CLO[tau_o])).all()

    base = np.zeros(TPC + 1, np.int64)
    base[1:] = np.cumsum(CLO + CHI)
    core_e = d_o // (TPC * P)
    j_e = d_o % P
    slot = (base[tau_o] + kchunk) * P + j_e

    chunk_is_lo = np.zeros(TOTC, bool)
    for t in range(TPC):
        chunk_is_lo[base[t]: base[t] + CLO[t]] = True

    idx_flat = np.where(chunk_is_lo[None, :, None], np.int16(LO_PAD),
                        np.int16(HI_PAD)).astype(np.int16)
    idx_flat = np.broadcast_to(idx_flat, (M, TOTC, P)).reshape(M, TOTC * P)
    idx_flat = np.ascontiguousarray(idx_flat)
    vals = np.where(is_lo_e, pi_src[eorder], pi_src[eorder] - HI_BASE)
    assert (vals >= 0).all() and (vals < WLEN).all()
    idx_flat[core_e, slot] = vals.astype(np.int16)

    lo_cids = np.flatnonzero(chunk_is_lo)
    hi_cids = np.flatnonzero(~chunk_is_lo)
    streams = {"lo": lo_cids, "hi": hi_cids}
    windows = []
    chunk_loc = {}
    col16 = 0
    for sname in ("lo", "hi"):
        cids = streams[sname]
        for wi0 in range(0, len(cids), WIN):
            wcids = cids[wi0: wi0 + WIN]
            swi = wi0 // WIN
            windows.append((sname, swi, len(wcids), col16))
            for sslot, cid in enumerate(wcids):
                chunk_loc[int(cid)] = (sname, swi, sslot)
            col16 += len(wcids) * P // 16
    TOT16 = col16

    idx_res = np.zeros((M, 128, TOT16), np.int16)
    for cc in range(M):
        for (sname, swi, nch, off) in windows:
            cids = streams[sname][swi * WIN: swi * WIN + nch]
            block = idx_flat[cc].reshape(TOTC, P)[cids].reshape(-1)
            wr = block.reshape(-1, 16).T
            idx_res[cc, :, off: off + nch * P // 16] = np.tile(wr, (8, 1))

    rdeg_pi = np.empty(NP, np.float32)
    rdeg_pi[pi] = (1.0 / np.maximum(deg, 1.0)).astype(np.float32)
    rdeg_ct = rdeg_pi.reshape(M, TPC, P).transpose(0, 2, 1)

    return dict(
        E=E, NP=NP, TPC=TPC, TOTC=TOTC, TOT16=TOT16, HI_BASE=HI_BASE,
        pi=pi, CLO=CLO, CHI=CHI, base=base,
        windows=windows, chunk_loc=chunk_loc,
        streams=streams, idx_res=idx_res,
        rdeg_ct=np.ascontiguousarray(rdeg_ct),
    )


# ---------------------------------------------------------------- program

def _build_program(s, D, DH, DO, repeat=1):
    NP, TPC, TOT16 = s["NP"], s["TPC"], s["TOT16"]
    CLO, CHI, base = s["CLO"], s["CHI"], s["base"]
    HI_BASE = s["HI_BASE"]
    windows, chunk_loc = s["windows"], s["chunk_loc"]
    NSH = TPC * P

    nc = bacc.Bacc("TRN2", target_bir_lowering=False, debug=False,
                   enable_asserts=False, num_devices=M, num_swdge_queues=4)

    x_full = nc.dram_tensor("x_full", [NP, D], F32, kind="ExternalInput")
    x_shard_t = nc.dram_tensor("x_shard_t", [P, TPC * D], F32,
                               kind="ExternalInput")
    idx_in = nc.dram_tensor("idx_in", [P, TOT16], I16, kind="ExternalInput")
    rdeg_in = nc.dram_tensor("rdeg_in", [P, TPC], F32, kind="ExternalInput")
    wcat1_in = nc.dram_tensor("wcat1_in", [D, D + 2], F32, kind="ExternalInput")
    wcat2_in = nc.dram_tensor("wcat2_in", [DH, DO + 2], F32,
                              kind="ExternalInput")
    params_in = nc.dram_tensor("params_in", [P, 2], F32, kind="ExternalInput")
    out_sh = nc.dram_tensor("out_sh", [NSH, DO], F32, kind="ExternalOutput")
    debug = os.environ.get("CC_GCN_DEBUG", "") == "1"
    if debug:
        dbg_h0 = nc.dram_tensor("dbg_h0", [NSH, D], F32, kind="ExternalOutput")
        dbg_acc = nc.dram_tensor("dbg_acc", [P, TPC * D], F32,
                                 kind="ExternalOutput")
        dbg_h1 = nc.dram_tensor("dbg_h1", [NSH, DH], F32, kind="ExternalOutput")

    RG = [list(range(M))]
    ROW1 = 2 * D  # conv1 table row width in BF16 elements: [X@W(D) | u | pad]

    with tile.TileContext(nc) as tc:
        with (
            tc.tile_pool(name="consts", bufs=1) as cp,
            tc.tile_pool(name="glo", bufs=3) as glop,
            tc.tile_pool(name="ghi", bufs=3) as ghip,
            tc.tile_pool(name="work", bufs=3) as wp,
            tc.tile_pool(name="small", bufs=4) as sp,
            tc.tile_pool(name="fpsum", bufs=3, space="PSUM") as fpp,
            tc.tile_pool(name="tpsum", bufs=2, space="PSUM") as tpp,
            tc.tile_pool(name="mpsum", bufs=2, space="PSUM") as mpp,
            tc.tile_pool(name="dram", bufs=1, space="DRAM") as dp,
        ):
            ident = cp.tile([P, P], F32, name="ident")
            make_identity(nc, ident[:])
            identb = cp.tile([P, P], BF16, name="identb")
            make_identity(nc, identb[:])
            idxt = cp.tile([P, TOT16], I16, name="idxt")
            nc.sync.dma_start(out=idxt[:], in_=idx_in[:])
            rdeg = cp.tile([P, TPC], F32, name="rdeg")
            nc.sync.dma_start(out=rdeg[:], in_=rdeg_in[:])
            wcat1 = cp.tile([D, D + 2], F32, name="wcat1")
            nc.sync.dma_start(out=wcat1[:], in_=wcat1_in[:])
            wcat2 = cp.tile([DH, DO + 2], F32, name="wcat2")
            nc.sync.dma_start(out=wcat2[:], in_=wcat2_in[:])
            params = cp.tile([P, 2], F32, name="params")
            nc.sync.dma_start(out=params[:], in_=params_in[:])
            acc = cp.tile([P, TPC * D], F32, name="acc")
            adst1 = cp.tile([P, TPC], F32, name="adst1")
            adst2 = cp.tile([P, TPC], F32, name="adst2")

            hin = dp.tile([NSH, D], F32, name="hin")
            t1in = dp.tile([NSH, ROW1], BF16, name="t1in")
            t2in = dp.tile([NSH, DH], F32, name="t2in")

            def emit_gathers(table_ap, drow, dtype, tag):
                bufs = {}
                qn = 0
                for (sname, swi, nch, off) in windows:
                    pool = glop if sname == "lo" else ghip
                    b = pool.tile([P, WIN * drow], dtype,
                                  name=f"g{tag}{sname}{swi}", tag=f"g{sname}")
                    num = nch * P
                    if sname == "lo":
                        src_ap = table_ap[0:min(WLEN, NP), :]
                    else:
                        src_ap = table_ap[HI_BASE:NP, :]
                    nc.gpsimd.dma_gather(
                        out_ap=b[:, : nch * drow].rearrange(
                            "p (c d) -> p c d", d=drow),
                        in_ap=src_ap,
                        idxs_ap=idxt[:, off: off + nch * P // 16],
                        num_idxs=num,
                        num_idxs_reg=num,
                        elem_size=drow,
                        single_packet=False,
                        queue_num=qn % 4,
                    )
                    qn += 1
                    bufs[(sname, swi)] = b
                return bufs

            def chunk_groups(t):
                runs = []
                for cid in range(int(base[t]), int(base[t + 1])):
                    sname, swi, sslot = chunk_loc[cid]
                    if runs and runs[-1][0] == (sname, swi) and \
                            runs[-1][1] + runs[-1][2] == sslot:
                        runs[-1] = (runs[-1][0], runs[-1][1], runs[-1][2] + 1)
                    else:
                        runs.append(((sname, swi), sslot, 1))
                return runs

            def proj_tile(t, xt_ap, wcat_t, din, dout, rowbuf_w, row_dt,
                          dest, adst_sb, bcol, tag):
                """rows [X@W | u]; saves a_dst column (+bias)."""
                tp = tpp.tile([din, P], F32, name=f"tp{tag}_{t}", tag="tps")
                nc.tensor.transpose(out=tp[:], in_=xt_ap, identity=ident[:])
                xT = sp.tile([din, P], F32, name=f"xT{tag}_{t}", tag="xT")
                nc.scalar.activation(out=xT[:], in_=tp[:],
                                     func=mybir.ActivationFunctionType.Copy)
                mp = mpp.tile([P, dout + 2], F32, name=f"mp{tag}_{t}",
                              tag="mps")
                nc.tensor.matmul(out=mp[:], lhsT=xT[:], rhs=wcat_t[:],
                                 start=True, stop=True)
                row = wp.tile([P, rowbuf_w], row_dt, name=f"row{tag}_{t}",
                              tag=f"row{tag}")
                nc.scalar.activation(out=row[:, : dout + 1],
                                     in_=mp[:, : dout + 1],
                                     func=mybir.ActivationFunctionType.Copy)
                nc.vector.tensor_scalar(
                    out=adst_sb[:, t:t + 1], in0=mp[:, dout + 1: dout + 2],
                    scalar1=bcol, scalar2=None, op0=mybir.AluOpType.add)
                nc.sync.dma_start(out=dest[t * P:(t + 1) * P, :], in_=row[:])

            def smoothing_pass(table_ap, pnum, rep, need_ag=True):
                bufs = emit_gathers(table_ap, D, F32, f"s{pnum}r{rep}")
                for t in range(TPC):
                    nch = int(CLO[t] + CHI[t])
                    h = sp.tile([P, D], F32, name=f"h{pnum}_{t}_{rep}",
                                tag="h")
                    if nch == 0:
                        nc.vector.memset(h[:], 0.0)
                    else:
                        ps = fpp.tile([P, D], F32, name=f"ps{pnum}_{t}_{rep}",
                                      tag="fps")
                        k = 0
                        for (bk, s0, n) in chunk_groups(t):
                            b = bufs[bk]
                            for si in range(s0, s0 + n):
                                nc.tensor.matmul(
                                    out=ps[:], lhsT=ident[:],
                                    rhs=b[:, si * D:(si + 1) * D],
                                    start=(k == 0), stop=(k == nch - 1))
                                k += 1
                        nc.vector.tensor_scalar(
                            out=h[:], in0=ps[:], scalar1=rdeg[:, t:t + 1],
                            scalar2=None, op0=mybir.AluOpType.mult)
                        nc.vector.tensor_tensor(
                            out=acc[:, t * D:(t + 1) * D],
                            in0=acc[:, t * D:(t + 1) * D], in1=h[:],
                            op=mybir.AluOpType.add)
                    if need_ag:
                        nc.sync.dma_start(out=hin[t * P:(t + 1) * P, :],
                                          in_=h[:])
                    if debug and pnum == 0:
                        nc.sync.dma_start(out=dbg_h0[t * P:(t + 1) * P, :],
                                          in_=h[:])

            def conv_pass(table_ap, drow, dtype, lhs_ident, df, adst_sb,
                          pnum, post_fn, rep):
                bufs = emit_gathers(table_ap, drow, dtype, f"c{pnum}r{rep}")
                for t in range(TPC):
                    nch = int(CLO[t] + CHI[t])
                    if nch == 0:
                        post_fn(t, None)
                        continue
                    ps = fpp.tile([P, df], F32, name=f"cp{pnum}_{t}_{rep}",
                                  tag="fps")
                    k = 0
                    for (bk, s0, n) in chunk_groups(t):
                        b = bufs[bk]
                        g3 = b[:, s0 * drow:(s0 + n) * drow].rearrange(
                            "p (c d) -> p c d", d=drow)
                        z = sp.tile([P, WIN], F32,
                                    name=f"z{pnum}_{t}_{k}_{rep}", tag="z")
                        nc.vector.tensor_scalar(
                            out=z[:, :n].rearrange("p (c u) -> p c u", u=1),
                            in0=g3[:, :, df:df + 1],
                            scalar1=adst_sb[:, t:t + 1], scalar2=None,
                            op0=mybir.AluOpType.add)
                        sc = sp.tile([P, WIN], F32,
                                     name=f"sc{pnum}_{t}_{k}_{rep}", tag="sc")
                        nc.scalar.activation(
                            out=sc[:, :n], in_=z[:, :n],
                            func=mybir.ActivationFunctionType.Lrelu,
                            alpha=NEG_SLOPE)
                        w8 = wp.tile([P, WIN * df], dtype,
                                     name=f"w8{pnum}_{t}_{k}_{rep}", tag="w8")
                        nc.vector.tensor_tensor(
                            out=w8[:, : n * df].rearrange(
                                "p (c d) -> p c d", d=df),
                            in0=g3[:, :, 0:df],
                            in1=sc[:, :n].to_broadcast([P, n, df]),
                            op=mybir.AluOpType.mult)
                        for si in range(n):
                            nc.tensor.matmul(
                                out=ps[:], lhsT=lhs_ident[:],
                                rhs=w8[:, si * df:(si + 1) * df],
                                start=(k == 0), stop=(k == nch - 1))
                            k += 1
                    post_fn(t, ps)

            for rep in range(repeat):
                htab1 = dp.tile([NP, D], F32, name=f"htab1_{rep}",
                                addr_space="Shared")
                htab2 = dp.tile([NP, D], F32, name=f"htab2_{rep}",
                                addr_space="Shared")
                t1tab = dp.tile([NP, ROW1], BF16, name=f"t1tab_{rep}",
                                addr_space="Shared")
                t2tab = dp.tile([NP, DH], F32, name=f"t2tab_{rep}",
                                addr_space="Shared")
                nc.sync.dma_start(out=acc[:], in_=x_shard_t[:])

                smoothing_pass(x_full.ap(), 0, rep)
                nc.gpsimd.collective_compute(
                    "AllGather", mybir.AluOpType.bypass,
                    ins=[hin.opt()], outs=[htab1.opt()], replica_groups=RG)
                smoothing_pass(htab1[:], 1, rep)
                nc.gpsimd.collective_compute(
                    "AllGather", mybir.AluOpType.bypass,
                    ins=[hin.opt()], outs=[htab2.opt()], replica_groups=RG)
                smoothing_pass(htab2[:], 2, rep, need_ag=False)

                if debug:
                    nc.sync.dma_start(out=dbg_acc[:], in_=acc[:])
                for t in range(TPC):
                    proj_tile(t, acc[:, t * D:(t + 1) * D], wcat1, D, D,
                              ROW1, BF16, t1in, adst1, params[:, 0:1],
                              f"t1_{rep}")
                nc.gpsimd.collective_compute(
                    "AllGather", mybir.AluOpType.bypass,
                    ins=[t1in.opt()], outs=[t1tab.opt()], replica_groups=RG)

                def post1(t, ps, rep=rep):
                    h1 = sp.tile([P, DH], F32, name=f"h1_{t}_{rep}", tag="h1")
                    if ps is None:
                        nc.vector.memset(h1[:], 0.0)
                    else:
                        nc.scalar.activation(
                            out=h1[:], in_=ps[:],
                            func=mybir.ActivationFunctionType.Relu)
                    if debug:
                        nc.sync.dma_start(out=dbg_h1[t * P:(t + 1) * P, :],
                                          in_=h1[:])
                    proj_tile(t, h1[:], wcat2, DH, DO, DH, F32, t2in, adst2,
                              params[:, 1:2], f"t2_{rep}")

                conv_pass(t1tab[:], ROW1, BF16, identb, D, adst1, 1,
                          post1, rep)
                nc.gpsimd.collective_compute(
                    "AllGather", mybir.AluOpType.bypass,
                    ins=[t2in.opt()], outs=[t2tab.opt()], replica_groups=RG)

                def post2(t, ps, rep=rep):
                    o = sp.tile([P, DO], F32, name=f"o_{t}_{rep}", tag="o")
                    if ps is None:
                        nc.vector.memset(o[:], 0.0)
                    else:
                        nc.scalar.activation(
                            out=o[:], in_=ps[:],
                            func=mybir.ActivationFunctionType.Copy)
                    nc.sync.dma_start(out=out_sh[t * P:(t + 1) * P, :],
                                      in_=o[:])

                conv_pass(t2tab[:], DH, F32, ident, DO, adst2, 2,
                          post2, rep)

    nc.compile()
    return nc


# ---------------------------------------------------------------- driver

_CACHE = {}
_SCHED_CACHE = {}


def _get_runner(s, D, DH, DO, repeat):
    key = (s["NP"], s["TOTC"], s["TOT16"], tuple(int(v) for v in s["CLO"]),
           tuple(int(v) for v in s["CHI"]), D, DH, DO, repeat)
    if key not in _CACHE:
        nc = _build_program(s, D, DH, DO, repeat)
        _CACHE[key] = _Runner(nc, M)
    return _CACHE[key]


def _prep_inputs(s, x, W_att1, b_att1, W_lin1, W_att2, b_att2, W_lin2):
    NP, TPC = s["NP"], s["TPC"]
    N, D = x.shape
    DH = W_lin1.shape[1]
    DO = W_lin2.shape[1]
    pi = s["pi"]

    x_full = np.zeros((NP, D), np.float32)
    x_full[pi[:N]] = x
    x_sh = x_full.reshape(M, TPC, P, D)

    wcat1 = np.concatenate(
        [W_lin1, W_att1[:D, :1], W_att1[D:, :1]], axis=1) * 0.25
    wcat2 = np.concatenate(
        [W_lin2, W_att2[:DH, :1], W_att2[DH:, :1]], axis=1)
    params = np.zeros((P, 2), np.float32)
    params[:, 0] = float(np.asarray(b_att1).reshape(-1)[0])
    params[:, 1] = float(np.asarray(b_att2).reshape(-1)[0])

    in_maps = []
    for c in range(M):
        in_maps.append({
            "x_full": x_full,
            "x_shard_t": np.ascontiguousarray(
                x_sh[c].transpose(1, 0, 2)).reshape(P, TPC * D),
            "idx_in": s["idx_res"][c],
            "rdeg_in": s["rdeg_ct"][c],
            "wcat1_in": wcat1.astype(np.float32),
            "wcat2_in": wcat2.astype(np.float32),
            "params_in": params,
        })
    return in_maps


def kernel(x, edge_index, W_att1, b_att1, W_lin1, W_att2, b_att2, W_lin2):
    x = np.asarray(x, np.float32)
    edge_index = np.asarray(edge_index)
    N, D = x.shape
    W_lin1 = np.asarray(W_lin1, np.float32)
    W_lin2 = np.asarray(W_lin2, np.float32)
    DH = W_lin1.shape[1]
    DO = W_lin2.shape[1]
    src = edge_index[0].astype(np.int64)
    dst = edge_index[1].astype(np.int64)

    s = _build_schedule(src, dst, N)
    repeat = int(os.environ.get("CC_GCN_REPEAT", "1"))
    r = _get_runner(s, D, DH, DO, repeat)
    in_maps = _prep_inputs(s, x, np.asarray(W_att1, np.float32),
                           np.asarray(b_att1, np.float32), W_lin1,
                           np.asarray(W_att2, np.float32),
                           np.asarray(b_att2, np.float32), W_lin2)
    res = r.run(in_maps)

    pi = s["pi"]
    out_pi = np.concatenate([res[c]["out_sh"] for c in range(M)], axis=0)
    return np.ascontiguousarray(out_pi[pi[:N]]).astype(np.float32)


# revision 8
# speedup vs baseline: 1.0691x; 1.0691x over previous
"""Trainium2 Bass kernel for nn_DeformableGCN (GNN message passing).

Strategy (1D graph partitioning over 8 NeuronCores):
  - Destination nodes are assigned to cores/tiles via a degree-sorted
    permutation pi: each 128-node tile holds nodes with nearly equal
    in-degree, so each dst node's in-edges occupy its own SBUF partition
    across a minimal number of 128-edge chunks ("identity scatter": the
    segment-sum matmul uses a constant identity weight matrix).
  - Per-edge source rows are fetched with the custom dma_gather
    instruction (int16 indices, 4 SWDGE queues). The node table is
    addressed through two OVERLAPPING 32768-row windows (lo = rows
    [0, 32768), hi = rows [NP-32768, NP)); sources in the overlap region
    may fill either stream, so a tile's chunk count approaches its max
    total in-degree instead of max-lo + max-hi. High-out-degree nodes
    are permuted into the overlap region to maximize flexible edges.
  - Each smoothing step computes the core's dst shard, then an AllGather
    rebuilds the full node table for the next step's gathers. The conv
    layers gather rows [h@W_lin | u_src] of per-node projected tables
    (AllGather'd); conv1's table is bf16 (a 256B descriptor carries the
    whole 65-value row vs 2x256B in f32), halving that pass's HBM and
    collective traffic. Edge scores lrelu(u_src + a_dst) use the
    per-tile a_dst column, partition-aligned by construction.
"""
import os
import sys

sys.path.insert(0, "/opt/trn_rl_repo")

import numpy as np

import concourse.bass as bass
import concourse.bacc as bacc
import concourse.mybir as mybir
import concourse.tile as tile
from concourse.masks import make_identity

M = 8            # cores
P = 128          # partitions
WLEN = 32768     # gather window length (int16-addressable rows)
WIN = 32         # gather-call window, in 128-edge chunks
F32 = mybir.dt.float32
BF16 = mybir.dt.bfloat16
I16 = mybir.dt.int16
NEG_SLOPE = 0.01


# ------------------------------------------------------------- pjrt runner

class _Runner:
    """Builds the jitted PJRT callable once; repeated exec without retrace."""

    def __init__(self, nc, n_cores):
        import jax
        from jax.sharding import Mesh, PartitionSpec
        from jax.experimental.shard_map import shard_map
        from concourse.bass2jax import (
            install_neuronx_cc_hook, _bass_exec_p, partition_id_tensor)
        install_neuronx_cc_hook()
        self.jax = jax
        self.n_cores = n_cores
        in_names, out_names, out_avals, zero_outs = [], [], [], []
        partition_name = (nc.partition_id_tensor.name
                          if nc.partition_id_tensor else None)
        for alloc in nc.m.functions[0].allocations:
            if not isinstance(alloc, mybir.MemoryLocationSet):
                continue
            name = alloc.memorylocations[0].name
            if alloc.kind == "ExternalInput":
                if name != partition_name:
                    in_names.append(name)
            elif alloc.kind == "ExternalOutput":
                shape = tuple(alloc.tensor_shape)
                dtype = mybir.dt.np(alloc.dtype)
                out_names.append(name)
                out_avals.append(jax.core.ShapedArray(shape, dtype))
                zero_outs.append(np.zeros(shape, dtype))
        self.in_names, self.out_names = in_names, out_names
        self.zero_outs = zero_outs
        n_params = len(in_names)
        all_in_names = list(in_names) + list(out_names)
        if partition_name is not None:
            all_in_names.append(partition_name)

        def _body(*args):
            operands = list(args)
            if partition_name is not None:
                operands.append(partition_id_tensor())
            outs = _bass_exec_p.bind(
                *operands,
                out_avals=tuple(out_avals),
                in_names=tuple(all_in_names),
                out_names=tuple(out_names),
                lowering_input_output_aliases=(),
                sim_require_finite=True,
                sim_require_nnan=True,
                nc=nc,
            )
            return tuple(outs)

        donate = tuple(range(n_params, n_params + len(out_names)))
        devices = jax.devices()[:n_cores]
        self.mesh = Mesh(np.asarray(devices), ("core",))
        in_specs = (PartitionSpec("core"),) * (n_params + len(out_names))
        out_specs = (PartitionSpec("core"),) * len(out_names)
        self.fn = jax.jit(
            shard_map(_body, mesh=self.mesh, in_specs=in_specs,
                      out_specs=out_specs, check_rep=False),
            donate_argnums=donate, keep_unused=True)
        self._dev_inputs = None

    def place_inputs(self, in_maps):
        import jax
        from jax.sharding import PartitionSpec, NamedSharding
        per_core = [[np.asarray(m[n]) for n in self.in_names]
                    for m in in_maps]
        arrs = []
        for i, n in enumerate(self.in_names):
            concat = np.concatenate(
                [per_core[c][i] for c in range(self.n_cores)], axis=0)
            arrs.append(jax.device_put(
                concat, NamedSharding(self.mesh, PartitionSpec("core"))))
        for a in arrs:
            a.block_until_ready()
        self._dev_inputs = arrs

    def _zeros(self):
        return [np.zeros((self.n_cores * z.shape[0], *z.shape[1:]), z.dtype)
                for z in self.zero_outs]

    def exec_async(self):
        return self.fn(*self._dev_inputs, *self._zeros())

    def run(self, in_maps=None):
        if in_maps is not None:
            self.place_inputs(in_maps)
        outs = [np.asarray(o) for o in self.exec_async()]
        res = []
        for c in range(self.n_cores):
            d = {}
            for i, n in enumerate(self.out_names):
                per = outs[i].reshape(
                    (self.n_cores, outs[i].shape[0] // self.n_cores)
                    + outs[i].shape[1:])
                d[n] = per[c]
            res.append(d)
        return res

    def time_exec(self, k=8):
        import time
        o = self.exec_async()
        self.jax.block_until_ready(o)
        t0 = time.perf_counter()
        outs = [self.exec_async() for _ in range(k)]
        self.jax.block_until_ready(outs)
        return (time.perf_counter() - t0) / k


# ---------------------------------------------------------------- schedule

def _build_schedule(src, dst, n_nodes, ls_iters=200000):
    """Host-side graph partitioning: permutation, slots, gather indices.

    Three source regions by final position: R0 = rows [0, NP-32768)
    (lo-stream only), R1 = [NP-32768, 32768) (either stream),
    R2 = [32768, NP) (hi-stream only).
    """
    E = src.shape[0]
    NP = -(-n_nodes // (M * P)) * (M * P)
    if NP - n_nodes < 2:
        NP += M * P
    TPC = NP // (M * P)
    NTILE = M * TPC
    HI_BASE = NP - WLEN
    assert 0 <= HI_BASE and NP <= 2 * WLEN and HI_BASE % P == 0
    n_r0_tiles = HI_BASE // P
    n_r1_tiles = (min(WLEN, NP) - HI_BASE) // P
    n_r2_tiles = NTILE - n_r0_tiles - n_r1_tiles

    deg = np.bincount(dst, minlength=NP).astype(np.int64)
    out_deg = np.bincount(src, minlength=NP).astype(np.int64)

    # region membership pinned before building the permutation; R1 (flex)
    # gets the highest OUT-degree nodes plus one fake (zero) pad node
    region = np.zeros(NP, np.int8)
    fakes = np.arange(n_nodes, NP)
    order_out = np.argsort(-out_deg, kind="stable")
    r1q = n_r1_tiles * P
    r1_sel = [i for i in order_out[: r1q] if i != fakes[0]][: r1q - 1]
    r1_sel.append(fakes[0])
    r1_mask = np.zeros(NP, bool)
    r1_mask[np.array(r1_sel)] = True
    assert r1_mask.sum() == r1q
    region[r1_mask] = 1
    rest = np.flatnonzero(~r1_mask)
    region[rest[: n_r0_tiles * P]] = 0
    region[rest[n_r0_tiles * P:]] = 2
    pad_fake = int(fakes[0])

    scat = region[src]            # 0=must-lo, 1=flex, 2=must-hi
    a = np.bincount(dst[scat == 0], minlength=NP)
    c = np.bincount(dst[scat == 1], minlength=NP)
    b = np.bincount(dst[scat == 2], minlength=NP)

    # tiles: total-degree primary, snake on a, then on b
    def pool_tiles(ids):
        t = deg[ids]
        sa = np.where(t % 2 == 0, a[ids], -a[ids])
        sb = np.where((t + a[ids]) % 2 == 0, b[ids], -b[ids])
        o = ids[np.lexsort((-sb, -sa, -t))]
        return o.reshape(-1, P)

    all_tiles = [(r, t) for r in (0, 1, 2)
                 for t in pool_tiles(np.flatnonzero(region == r))]
    kinds = np.array([r for r, _ in all_tiles])
    mtot = np.array([deg[t].max() for _, t in all_tiles])
    ma = np.array([a[t].max() for _, t in all_tiles])
    mb = np.array([b[t].max() for _, t in all_tiles])
    tcost = np.maximum(mtot, ma + mb)

    # position slots (c, tau) carry fixed region labels
    pos_region = np.zeros(NTILE, np.int8)
    pos_region[n_r0_tiles: n_r0_tiles + n_r1_tiles] = 1
    pos_region[n_r0_tiles + n_r1_tiles:] = 2
    quota = [[[] for _ in range(TPC)] for _ in range(3)]
    for pos in range(NTILE):
        cc, tau = divmod(pos, TPC)
        quota[pos_region[pos]][tau].append(cc)

    def tau_cost(mt, mA, mB):
        return max(mt, mA + mB)

    # percentile-aligned construction
    ids_sorted = {r: list(np.flatnonzero(kinds == r)[
        np.argsort(-tcost[kinds == r], kind="stable")]) for r in (0, 1, 2)}
    members = [[[] for _ in range(TPC)] for _ in range(3)]
    ptr = {0: 0, 1: 0, 2: 0}
    for tau in range(TPC):
        for r in (0, 1, 2):
            n = len(quota[r][tau])
            members[r][tau] = ids_sorted[r][ptr[r]: ptr[r] + n]
            ptr[r] += n

    def pos_cost(t):
        tis = members[0][t] + members[1][t] + members[2][t]
        mt = max((mtot[i] for i in tis), default=0)
        mA = max((ma[i] for i in tis), default=0)
        mB = max((mb[i] for i in tis), default=0)
        return tau_cost(mt, mA, mB)

    # simulated annealing over same-region tile swaps
    rng = np.random.default_rng(0)
    cur = np.array([pos_cost(t) for t in range(TPC)], np.int64)
    T0, T1 = 2.0, 0.01
    for it in range(ls_iters):
        temp = T0 * (T1 / T0) ** (it / max(ls_iters - 1, 1))
        k = int(rng.integers(0, 3))
        p_, q_ = (int(v) for v in rng.integers(0, TPC, 2))
        if p_ == q_ or not members[k][p_] or not members[k][q_]:
            continue
        i = members[k][p_][int(rng.integers(len(members[k][p_])))]
        j = members[k][q_][int(rng.integers(len(members[k][q_])))]
        before = cur[p_] + cur[q_]
        members[k][p_].remove(i)
        members[k][q_].remove(j)
        members[k][p_].append(j)
        members[k][q_].append(i)
        np_, nq_ = pos_cost(p_), pos_cost(q_)
        d = (np_ + nq_) - before
        if d <= 0 or rng.random() < np.exp(-d / temp):
            cur[p_], cur[q_] = np_, nq_
        else:
            members[k][p_].remove(j)
            members[k][q_].remove(i)
            members[k][p_].append(i)
            members[k][q_].append(j)

    CLO = np.zeros(TPC, np.int64)
    CHI = np.zeros(TPC, np.int64)
    for tau in range(TPC):
        tis = members[0][tau] + members[1][tau] + members[2][tau]
        mt = max((mtot[i] for i in tis), default=0)
        mA = max((ma[i] for i in tis), default=0)
        mB = max((mb[i] for i in tis), default=0)
        cost = tau_cost(mt, mA, mB)
        CLO[tau] = cost - mB
        CHI[tau] = mB
        assert CLO[tau] >= mA
    TOTC = int((CLO + CHI).sum())

    pi = np.empty(NP, np.int64)
    for r in (0, 1, 2):
        for tau in range(TPC):
            cores = list(quota[r][tau])
            for ti in members[r][tau]:
                cc = cores.pop()
                pi[all_tiles[ti][1]] = (cc * TPC + tau) * P + np.arange(P)
    pi_src = pi[src]
    pi_dst = pi[dst]

    LO_PAD = int(pi[pad_fake])
    HI_PAD = LO_PAD - HI_BASE
    assert 0 <= LO_PAD < WLEN and 0 <= HI_PAD < WLEN

    # per-edge stream + chunk-slot assignment
    cat = scat.astype(np.int64)
    tau_e = (pi_dst % (TPC * P)) // P
    key = pi_dst * 3 + cat
    eorder = np.argsort(key, kind="stable")
    ks = key[eorder]
    new_grp = np.ones(E, bool)
    new_grp[1:] = ks[1:] != ks[:-1]
    starts = np.flatnonzero(new_grp)
    grp_id = np.cumsum(new_grp) - 1
    rank = np.arange(E) - starts[grp_id]

    d_o = pi_dst[eorder]
    cat_o = cat[eorder]
    tau_o = tau_e[eorder]
    a_o = a[dst[eorder]]
    c_o = c[dst[eorder]]
    x_o = np.minimum(c_o, CLO[tau_o] - a_o)   # flex edges sent to lo
    is_lo_e = (cat_o == 0) | ((cat_o == 1) & (rank < x_o))
    lo_rank = np.where(cat_o == 0, rank, a_o + rank)
    hi_rank = np.where(cat_o == 1, rank - x_o,
                       np.maximum(c_o - x_o, 0) + rank)
    kchunk = np.where(is_lo_e, lo_rank, CLO[tau_o] + hi_rank)
    assert (np.where(is_lo_e, kchunk < CLO[tau_o],
                     kchunk < CLO[tau_o] + CHI[tau_o])).all()
    assert (kchunk >= np.where(is_lo_e, 0, CLO[tau_o])).all()

    base = np.zeros(TPC + 1, np.int64)
    base[1:] = np.cumsum(CLO + CHI)
    core_e = d_o // (TPC * P)
    j_e = d_o % P
    slot = (base[tau_o] + kchunk) * P + j_e

    chunk_is_lo = np.zeros(TOTC, bool)
    for t in range(TPC):
        chunk_is_lo[base[t]: base[t] + CLO[t]] = True

    idx_flat = np.where(chunk_is_lo[None, :, None], np.int16(LO_PAD),
                        np.int16(HI_PAD)).astype(np.int16)
    idx_flat = np.broadcast_to(idx_flat, (M, TOTC, P)).reshape(M, TOTC * P)
    idx_flat = np.ascontiguousarray(idx_flat)
    vals = np.where(is_lo_e, pi_src[eorder], pi_src[eorder] - HI_BASE)
    assert (vals >= 0).all() and (vals < WLEN).all()
    idx_flat[core_e, slot] = vals.astype(np.int16)

    lo_cids = np.flatnonzero(chunk_is_lo)
    hi_cids = np.flatnonzero(~chunk_is_lo)
    streams = {"lo": lo_cids, "hi": hi_cids}
    windows = []
    chunk_loc = {}
    col16 = 0
    for sname in ("lo", "hi"):
        cids = streams[sname]
        for wi0 in range(0, len(cids), WIN):
            wcids = cids[wi0: wi0 + WIN]
            swi = wi0 // WIN
            windows.append((sname, swi, len(wcids), col16))
            for sslot, cid in enumerate(wcids):
                chunk_loc[int(cid)] = (sname, swi, sslot)
            col16 += len(wcids) * P // 16
    TOT16 = col16

    idx_res = np.zeros((M, 128, TOT16), np.int16)
    for cc in range(M):
        for (sname, swi, nch, off) in windows:
            cids = streams[sname][swi * WIN: swi * WIN + nch]
            block = idx_flat[cc].reshape(TOTC, P)[cids].reshape(-1)
            wr = block.reshape(-1, 16).T
            idx_res[cc, :, off: off + nch * P // 16] = np.tile(wr, (8, 1))

    rdeg_pi = np.empty(NP, np.float32)
    rdeg_pi[pi] = (1.0 / np.maximum(deg, 1.0)).astype(np.float32)
    rdeg_ct = rdeg_pi.reshape(M, TPC, P).transpose(0, 2, 1)

    return dict(
        E=E, NP=NP, TPC=TPC, TOTC=TOTC, TOT16=TOT16, HI_BASE=HI_BASE,
        pi=pi, CLO=CLO, CHI=CHI, base=base,
        windows=windows, chunk_loc=chunk_loc,
        streams=streams, idx_res=idx_res,
        rdeg_ct=np.ascontiguousarray(rdeg_ct),
    )


# ---------------------------------------------------------------- program

def _build_program(s, D, DH, DO, repeat=1, queue_map=None):
    NP, TPC, TOT16 = s["NP"], s["TPC"], s["TOT16"]
    CLO, CHI, base = s["CLO"], s["CHI"], s["base"]
    HI_BASE = s["HI_BASE"]
    windows, chunk_loc = s["windows"], s["chunk_loc"]
    NSH = TPC * P

    nc = bacc.Bacc("TRN2", target_bir_lowering=False, debug=False,
                   enable_asserts=False, num_devices=M, num_swdge_queues=4)

    x_full = nc.dram_tensor("x_full", [NP, D], F32, kind="ExternalInput")
    x_shard_t = nc.dram_tensor("x_shard_t", [P, TPC * D], F32,
                               kind="ExternalInput")
    idx_in = nc.dram_tensor("idx_in", [P, TOT16], I16, kind="ExternalInput")
    rdeg_in = nc.dram_tensor("rdeg_in", [P, TPC], F32, kind="ExternalInput")
    wcat1_in = nc.dram_tensor("wcat1_in", [D, D + 2], F32, kind="ExternalInput")
    wcat2_in = nc.dram_tensor("wcat2_in", [DH, DO + 2], F32,
                              kind="ExternalInput")
    params_in = nc.dram_tensor("params_in", [P, 2], F32, kind="ExternalInput")
    out_sh = nc.dram_tensor("out_sh", [NSH, DO], F32, kind="ExternalOutput")
    debug = os.environ.get("CC_GCN_DEBUG", "") == "1"
    if debug:
        dbg_h0 = nc.dram_tensor("dbg_h0", [NSH, D], F32, kind="ExternalOutput")
        dbg_acc = nc.dram_tensor("dbg_acc", [P, TPC * D], F32,
                                 kind="ExternalOutput")
        dbg_h1 = nc.dram_tensor("dbg_h1", [NSH, DH], F32, kind="ExternalOutput")

    RG = [list(range(M))]
    ROW1 = 2 * D  # conv1 table row width in BF16 elements: [X@W(D) | u | pad]

    with tile.TileContext(nc) as tc:
        with (
            tc.tile_pool(name="consts", bufs=1) as cp,
            tc.tile_pool(name="glo", bufs=3) as glop,
            tc.tile_pool(name="ghi", bufs=3) as ghip,
            tc.tile_pool(name="work", bufs=3) as wp,
            tc.tile_pool(name="small", bufs=4) as sp,
            tc.tile_pool(name="fpsum", bufs=3, space="PSUM") as fpp,
            tc.tile_pool(name="tpsum", bufs=2, space="PSUM") as tpp,
            tc.tile_pool(name="mpsum", bufs=2, space="PSUM") as mpp,
            tc.tile_pool(name="dram", bufs=1, space="DRAM") as dp,
        ):
            ident = cp.tile([P, P], F32, name="ident")
            make_identity(nc, ident[:])
            identb = cp.tile([P, P], BF16, name="identb")
            make_identity(nc, identb[:])
            idxt = cp.tile([P, TOT16], I16, name="idxt")
            nc.sync.dma_start(out=idxt[:], in_=idx_in[:])
            rdeg = cp.tile([P, TPC], F32, name="rdeg")
            nc.sync.dma_start(out=rdeg[:], in_=rdeg_in[:])
            wcat1 = cp.tile([D, D + 2], F32, name="wcat1")
            nc.sync.dma_start(out=wcat1[:], in_=wcat1_in[:])
            wcat2 = cp.tile([DH, DO + 2], F32, name="wcat2")
            nc.sync.dma_start(out=wcat2[:], in_=wcat2_in[:])
            params = cp.tile([P, 2], F32, name="params")
            nc.sync.dma_start(out=params[:], in_=params_in[:])
            acc = cp.tile([P, TPC * D], F32, name="acc")
            adst1 = cp.tile([P, TPC], F32, name="adst1")
            adst2 = cp.tile([P, TPC], F32, name="adst2")

            hin = dp.tile([NSH, D], F32, name="hin")
            t1in = dp.tile([NSH, ROW1], BF16, name="t1in")
            t2in = dp.tile([NSH, DH], F32, name="t2in")

            # The Tile framework assigns DMASW sem lanes round-robin (mod 8)
            # over Pool-engine DMA instructions in SCHEDULED order; each sem
            # lane must stay bound to a single SWDGE queue (queue = lane % 4)
            # or consumers over-synchronize across queues. The scheduled
            # order is only known post-compile, so _get_runner builds twice:
            # pass 1 records each gather's lane, pass 2 replays with
            # queue_map[i] = lane_of_emission_i % 4.
            gq = [0]

            def emit_gathers(table_ap, drow, dtype, tag):
                bufs = {}
                for (sname, swi, nch, off) in windows:
                    pool = glop if sname == "lo" else ghip
                    b = pool.tile([P, WIN * drow], dtype,
                                  name=f"g{tag}{sname}{swi}", tag=f"g{sname}")
                    num = nch * P
                    if sname == "lo":
                        src_ap = table_ap[0:min(WLEN, NP), :]
                    else:
                        src_ap = table_ap[HI_BASE:NP, :]
                    nc.gpsimd.dma_gather(
                        out_ap=b[:, : nch * drow].rearrange(
                            "p (c d) -> p c d", d=drow),
                        in_ap=src_ap,
                        idxs_ap=idxt[:, off: off + nch * P // 16],
                        num_idxs=num,
                        num_idxs_reg=num,
                        elem_size=drow,
                        single_packet=False,
                        queue_num=(queue_map[gq[0]] if queue_map is not None
                                   else gq[0] % 4),
                    )
                    gq[0] += 1
                    bufs[(sname, swi)] = b
                return bufs

            def chunk_groups(t):
                runs = []
                for cid in range(int(base[t]), int(base[t + 1])):
                    sname, swi, sslot = chunk_loc[cid]
                    if runs and runs[-1][0] == (sname, swi) and \
                            runs[-1][1] + runs[-1][2] == sslot:
                        runs[-1] = (runs[-1][0], runs[-1][1], runs[-1][2] + 1)
                    else:
                        runs.append(((sname, swi), sslot, 1))
                return runs

            def proj_tile(t, xt_ap, wcat_t, din, dout, rowbuf_w, row_dt,
                          dest, adst_sb, bcol, tag):
                """rows [X@W | u]; saves a_dst column (+bias)."""
                tp = tpp.tile([din, P], F32, name=f"tp{tag}_{t}", tag="tps")
                nc.tensor.transpose(out=tp[:], in_=xt_ap, identity=ident[:])
                xT = sp.tile([din, P], F32, name=f"xT{tag}_{t}", tag="xT")
                nc.scalar.activation(out=xT[:], in_=tp[:],
                                     func=mybir.ActivationFunctionType.Copy)
                mp = mpp.tile([P, dout + 2], F32, name=f"mp{tag}_{t}",
                              tag="mps")
                nc.tensor.matmul(out=mp[:], lhsT=xT[:], rhs=wcat_t[:],
                                 start=True, stop=True)
                row = wp.tile([P, rowbuf_w], row_dt, name=f"row{tag}_{t}",
                              tag=f"row{tag}")
                nc.scalar.activation(out=row[:, : dout + 1],
                                     in_=mp[:, : dout + 1],
                                     func=mybir.ActivationFunctionType.Copy)
                nc.vector.tensor_scalar(
                    out=adst_sb[:, t:t + 1], in0=mp[:, dout + 1: dout + 2],
                    scalar1=bcol, scalar2=None, op0=mybir.AluOpType.add)
                nc.sync.dma_start(out=dest[t * P:(t + 1) * P, :], in_=row[:])

            def smoothing_pass(table_ap, pnum, rep, need_ag=True):
                bufs = emit_gathers(table_ap, D, F32, f"s{pnum}r{rep}")
                for t in range(TPC):
                    nch = int(CLO[t] + CHI[t])
                    h = sp.tile([P, D], F32, name=f"h{pnum}_{t}_{rep}",
                                tag="h")
                    if nch == 0:
                        nc.vector.memset(h[:], 0.0)
                    else:
                        ps = fpp.tile([P, D], F32, name=f"ps{pnum}_{t}_{rep}",
                                      tag="fps")
                        k = 0
                        for (bk, s0, n) in chunk_groups(t):
                            b = bufs[bk]
                            for si in range(s0, s0 + n):
                                nc.tensor.matmul(
                                    out=ps[:], lhsT=ident[:],
                                    rhs=b[:, si * D:(si + 1) * D],
                                    start=(k == 0), stop=(k == nch - 1))
                                k += 1
                        nc.vector.tensor_scalar(
                            out=h[:], in0=ps[:], scalar1=rdeg[:, t:t + 1],
                            scalar2=None, op0=mybir.AluOpType.mult)
                        nc.vector.tensor_tensor(
                            out=acc[:, t * D:(t + 1) * D],
                            in0=acc[:, t * D:(t + 1) * D], in1=h[:],
                            op=mybir.AluOpType.add)
                    if need_ag:
                        nc.sync.dma_start(out=hin[t * P:(t + 1) * P, :],
                                          in_=h[:])
                    if debug and pnum == 0:
                        nc.sync.dma_start(out=dbg_h0[t * P:(t + 1) * P, :],
                                          in_=h[:])

            def conv_pass(table_ap, drow, dtype, lhs_ident, df, adst_sb,
                          pnum, post_fn, rep):
                bufs = emit_gathers(table_ap, drow, dtype, f"c{pnum}r{rep}")
                for t in range(TPC):
                    nch = int(CLO[t] + CHI[t])
                    if nch == 0:
                        post_fn(t, None)
                        continue
                    ps = fpp.tile([P, df], F32, name=f"cp{pnum}_{t}_{rep}",
                                  tag="fps")
                    k = 0
                    for (bk, s0, n) in chunk_groups(t):
                        b = bufs[bk]
                        g3 = b[:, s0 * drow:(s0 + n) * drow].rearrange(
                            "p (c d) -> p c d", d=drow)
                        z = sp.tile([P, WIN], F32,
                                    name=f"z{pnum}_{t}_{k}_{rep}", tag="z")
                        nc.vector.tensor_scalar(
                            out=z[:, :n].rearrange("p (c u) -> p c u", u=1),
                            in0=g3[:, :, df:df + 1],
                            scalar1=adst_sb[:, t:t + 1], scalar2=None,
                            op0=mybir.AluOpType.add)
                        sc = sp.tile([P, WIN], F32,
                                     name=f"sc{pnum}_{t}_{k}_{rep}", tag="sc")
                        nc.scalar.activation(
                            out=sc[:, :n], in_=z[:, :n],
                            func=mybir.ActivationFunctionType.Lrelu,
                            alpha=NEG_SLOPE)
                        w8 = wp.tile([P, WIN * df], dtype,
                                     name=f"w8{pnum}_{t}_{k}_{rep}", tag="w8")
                        nc.vector.tensor_tensor(
                            out=w8[:, : n * df].rearrange(
                                "p (c d) -> p c d", d=df),
                            in0=g3[:, :, 0:df],
                            in1=sc[:, :n].to_broadcast([P, n, df]),
                            op=mybir.AluOpType.mult)
                        for si in range(n):
                            nc.tensor.matmul(
                                out=ps[:], lhsT=lhs_ident[:],
                                rhs=w8[:, si * df:(si + 1) * df],
                                start=(k == 0), stop=(k == nch - 1))
                            k += 1
                    post_fn(t, ps)

            for rep in range(repeat):
                htab1 = dp.tile([NP, D], F32, name=f"htab1_{rep}",
                                addr_space="Shared")
                htab2 = dp.tile([NP, D], F32, name=f"htab2_{rep}",
                                addr_space="Shared")
                t1tab = dp.tile([NP, ROW1], BF16, name=f"t1tab_{rep}",
                                addr_space="Shared")
                t2tab = dp.tile([NP, DH], F32, name=f"t2tab_{rep}",
                                addr_space="Shared")
                nc.sync.dma_start(out=acc[:], in_=x_shard_t[:])

                smoothing_pass(x_full.ap(), 0, rep)
                nc.gpsimd.collective_compute(
                    "AllGather", mybir.AluOpType.bypass,
                    ins=[hin.opt()], outs=[htab1.opt()], replica_groups=RG)
                smoothing_pass(htab1[:], 1, rep)
                nc.gpsimd.collective_compute(
                    "AllGather", mybir.AluOpType.bypass,
                    ins=[hin.opt()], outs=[htab2.opt()], replica_groups=RG)
                smoothing_pass(htab2[:], 2, rep, need_ag=False)

                if debug:
                    nc.sync.dma_start(out=dbg_acc[:], in_=acc[:])
                for t in range(TPC):
                    proj_tile(t, acc[:, t * D:(t + 1) * D], wcat1, D, D,
                              ROW1, BF16, t1in, adst1, params[:, 0:1],
                              f"t1_{rep}")
                nc.gpsimd.collective_compute(
                    "AllGather", mybir.AluOpType.bypass,
                    ins=[t1in.opt()], outs=[t1tab.opt()], replica_groups=RG)

                def post1(t, ps, rep=rep):
                    h1 = sp.tile([P, DH], F32, name=f"h1_{t}_{rep}", tag="h1")
                    if ps is None:
                        nc.vector.memset(h1[:], 0.0)
                    else:
                        nc.scalar.activation(
                            out=h1[:], in_=ps[:],
                            func=mybir.ActivationFunctionType.Relu)
                    if debug:
                        nc.sync.dma_start(out=dbg_h1[t * P:(t + 1) * P, :],
                                          in_=h1[:])
                    proj_tile(t, h1[:], wcat2, DH, DO, DH, F32, t2in, adst2,
                              params[:, 1:2], f"t2_{rep}")

                conv_pass(t1tab[:], ROW1, BF16, identb, D, adst1, 1,
                          post1, rep)
                nc.gpsimd.collective_compute(
                    "AllGather", mybir.AluOpType.bypass,
                    ins=[t2in.opt()], outs=[t2tab.opt()], replica_groups=RG)

                def post2(t, ps, rep=rep):
                    o = sp.tile([P, DO], F32, name=f"o_{t}_{rep}", tag="o")
                    if ps is None:
                        nc.vector.memset(o[:], 0.0)
                    else:
                        nc.scalar.activation(
                            out=o[:], in_=ps[:],
                            func=mybir.ActivationFunctionType.Copy)
                    nc.sync.dma_start(out=out_sh[t * P:(t + 1) * P, :],
                                      in_=o[:])

                conv_pass(t2tab[:], DH, F32, ident, DO, adst2, 2,
                          post2, rep)

    nc.compile()
    return nc


# ---------------------------------------------------------------- driver

_CACHE = {}
_SCHED_CACHE = {}


def _gather_lanes(nc):
    """Per-gather DMASW lane (in emission order) from the scheduled IR."""
    gathers = []
    for name, inst in nc.inst_map.items():
        if isinstance(inst, mybir.InstDMAGatherAnt):
            gathers.append((int(name.split("-")[1]), inst))
    gathers.sort()
    from concourse.tile_sem_assignment import PROC_NAME_TO_IDX
    idx_to_proc = {v: k for k, v in PROC_NAME_TO_IDX.items()}
    lanes = []
    for _, inst in gathers:
        lane = idx_to_proc.get(inst.bass_scheduled_proc, "")
        lanes.append(int(lane[5:]) % 4 if lane.startswith("DMASW") else 0)
    return lanes, [inst.queue_num for _, inst in gathers]


def _build_aligned(s, D, DH, DO, repeat):
    queue_map = None
    nc = None
    for _ in range(3):
        nc = _build_program(s, D, DH, DO, repeat, queue_map=queue_map)
        lanes, queues = _gather_lanes(nc)
        if lanes == queues:
            break
        queue_map = lanes
    return nc


def _get_runner(s, D, DH, DO, repeat):
    key = (s["NP"], s["TOTC"], s["TOT16"], tuple(int(v) for v in s["CLO"]),
           tuple(int(v) for v in s["CHI"]), D, DH, DO, repeat)
    if key not in _CACHE:
        nc = _build_aligned(s, D, DH, DO, repeat)
        _CACHE[key] = _Runner(nc, M)
    return _CACHE[key]


def _prep_inputs(s, x, W_att1, b_att1, W_lin1, W_att2, b_att2, W_lin2):
    NP, TPC = s["NP"], s["TPC"]
    N, D = x.shape
    DH = W_lin1.shape[1]
    DO = W_lin2.shape[1]
    pi = s["pi"]

    x_full = np.zeros((NP, D), np.float32)
    x_full[pi[:N]] = x
    x_sh = x_full.reshape(M, TPC, P, D)

    wcat1 = np.concatenate(
        [W_lin1, W_att1[:D, :1], W_att1[D:, :1]], axis=1) * 0.25
    wcat2 = np.concatenate(
        [W_lin2, W_att2[:DH, :1], W_att2[DH:, :1]], axis=1)
    params = np.zeros((P, 2), np.float32)
    params[:, 0] = float(np.asarray(b_att1).reshape(-1)[0])
    params[:, 1] = float(np.asarray(b_att2).reshape(-1)[0])

    in_maps = []
    for c in range(M):
        in_maps.append({
            "x_full": x_full,
            "x_shard_t": np.ascontiguousarray(
                x_sh[c].transpose(1, 0, 2)).reshape(P, TPC * D),
            "idx_in": s["idx_res"][c],
            "rdeg_in": s["rdeg_ct"][c],
            "wcat1_in": wcat1.astype(np.float32),
            "wcat2_in": wcat2.astype(np.float32),
            "params_in": params,
        })
    return in_maps


def kernel(x, edge_index, W_att1, b_att1, W_lin1, W_att2, b_att2, W_lin2):
    x = np.asarray(x, np.float32)
    edge_index = np.asarray(edge_index)
    N, D = x.shape
    W_lin1 = np.asarray(W_lin1, np.float32)
    W_lin2 = np.asarray(W_lin2, np.float32)
    DH = W_lin1.shape[1]
    DO = W_lin2.shape[1]
    src = edge_index[0].astype(np.int64)
    dst = edge_index[1].astype(np.int64)

    s = _build_schedule(src, dst, N)
    repeat = int(os.environ.get("CC_GCN_REPEAT", "1"))
    r = _get_runner(s, D, DH, DO, repeat)
    in_maps = _prep_inputs(s, x, np.asarray(W_att1, np.float32),
                           np.asarray(b_att1, np.float32), W_lin1,
                           np.asarray(W_att2, np.float32),
                           np.asarray(b_att2, np.float32), W_lin2)
    res = r.run(in_maps)

    pi = s["pi"]
    out_pi = np.concatenate([res[c]["out_sh"] for c in range(M)], axis=0)
    return np.ascontiguousarray(out_pi[pi[:N]]).astype(np.float32)
